# revision 1
# baseline (speedup 1.0000x reference)
"""MoFE (mixture of depthwise-conv experts) Trainium2 kernel.

Full inputs in, full outputs out; internally sharded data-parallel over the
batch dim across 8 NeuronCores (B=8, one sample per core).

Per-core program (Bass/Tile):
  pass A: stream x, per-strip max/sum reduces -> gate (pooled -> fc -> noisy
          top-k softmax coefficients, all on device)
  pass B: per strip: depthwise conv1 (+bias, relu), depthwise conv2,
          cof-weighted accumulation over experts, store.
"""

import numpy as np

import concourse.bass as bass
import concourse.tile as tile
from concourse import mybir
from concourse.bass_utils import run_bass_kernel_spmd

F32 = mybir.dt.float32
AX = mybir.AxisListType if hasattr(mybir, "AxisListType") else None
ALU = mybir.AluOpType
ACT = mybir.ActivationFunctionType

B, C, H, W = 8, 96, 192, 192
E = 6
N_CORES = 8
TH = 24                      # strip height (output rows per strip)
NS = H // TH                 # strips
TAPS = [(ky - 1, kx - 1) for ky in range(3) for kx in range(3)]


# ---------------------------------------------------------------------------
# walrus workaround: split instructions carrying >maxw semaphore waits
# ---------------------------------------------------------------------------
def _split_multiwait(nc, maxw: int = 1) -> int:
    n_split = 0
    for f in nc.m.functions:
        for b in f.blocks:
            insts = b.instructions
            new_list = []
            changed = False
            for inst in insts:
                si = getattr(inst, "sync_info", None)
                waits = list(si.on_wait) if (si and si.on_wait) else []
                if len(waits) > maxw:
                    changed = True
                    chunks = [waits[j: j + maxw] for j in range(0, len(waits), maxw)]
                    for k, ch in enumerate(chunks[:-1]):
                        nop = mybir.InstNoOp(
                            name=f"{inst.name}_wsplit{k}",
                            sync_info=mybir.SyncInfo(on_wait=ch, on_update=[]),
                            bass_nofuse=True,
                            engine=inst.engine,
                        )
                        new_list.append(nop)
                        n_split += 1
                    si.on_wait = chunks[-1]
                new_list.append(inst)
            if changed:
                if isinstance(insts, list):
                    insts[:] = new_list
                else:
                    b.instructions = new_list
    return n_split


# ---------------------------------------------------------------------------
# device program
# ---------------------------------------------------------------------------
BF = mybir.dt.bfloat16
K = 3            # top-k slots
WP = W + 2       # padded width
GUARD = 200      # flat guard elems around the resident padded image
CHUNK = 512      # psum bank free size (f32)
CONF_C1_GP = {0: 0, 1: 0, 2: 0}
CONF_C2_GP = {0: 3, 1: 3, 2: 3}
CONF_C2_ACT = {0: 0, 1: 0, 2: 0}


def _build(split: bool = True):
    nc = bass.Bass()
    x = nc.declare_dram_parameter("x", [C, H, W], F32, isOutput=False)
    wfc = nc.declare_dram_parameter("wfc", [C, 2 * E], F32, isOutput=False)
    bfc = nc.declare_dram_parameter("bfc", [1, 2 * E], F32, isOutput=False)
    w1 = nc.declare_dram_parameter("w1", [C, E * 9], F32, isOutput=False)
    b1 = nc.declare_dram_parameter("b1", [C, E], F32, isOutput=False)
    w2 = nc.declare_dram_parameter("w2", [C, E * 9], F32, isOutput=False)
    b2 = nc.declare_dram_parameter("b2", [C, E], F32, isOutput=False)
    eye = nc.declare_dram_parameter("eye", [C, C], F32, isOutput=False)
    y = nc.declare_dram_parameter("y", [C, H, W], F32, isOutput=True)

    v = nc.vector
    g = nc.gpsimd
    sc = nc.scalar
    sy = nc.sync

    with tile.TileContext(nc) as tc:
        with (
            tc.tile_pool(name="const", bufs=1) as cpool,
            tc.tile_pool(name="gate", bufs=1) as gpool,
            tc.tile_pool(name="xa", bufs=2) as xa_pool,
            tc.tile_pool(name="hbuf", bufs=2) as h_pool,
            tc.tile_pool(name="h2buf", bufs=1) as h2_pool,
            tc.tile_pool(name="oacc", bufs=2) as oacc_pool,
            tc.tile_pool(name="of32", bufs=1) as of32_pool,
            tc.tile_pool(name="og", bufs=1) as og_pool,
            tc.tile_pool(name="ps", bufs=1, space="PSUM") as ps_pool,
            tc.tile_pool(name="psc", bufs=6, space="PSUM") as psc_pool,
        ):
            # ---- constants ------------------------------------------------
            w1_sb = cpool.tile([C, E * 9], F32)
            sy.dma_start(w1_sb[:], w1[:])
            b1_sb = cpool.tile([C, E], F32)
            sy.dma_start(b1_sb[:], b1[:])
            w2_sb = cpool.tile([C, E * 9], F32)
            sy.dma_start(w2_sb[:], w2[:])
            b2_sb = cpool.tile([C, E], F32)
            sy.dma_start(b2_sb[:], b2[:])
            wfc_sb = cpool.tile([C, 2 * E], F32)
            sy.dma_start(wfc_sb[:], wfc[:])
            bfc_sb = cpool.tile([1, 2 * E], F32)
            sy.dma_start(bfc_sb[:], bfc[:])
            eye_sb = cpool.tile([C, C], F32)
            sy.dma_start(eye_sb[:], eye[:])
            ones96 = cpool.tile([1, C], F32)
            g.memset(ones96[:], 1.0)

            # resident zero-padded bf16 image, flat with guards
            NPAD = WP * WP
            xpg = cpool.tile([C, GUARD + NPAD + GUARD], BF)
            # zero only guards + padding (a full memset would cost ~40us and
            # serialize the head): front guard + top pad row, bottom pad row +
            # back guard, and the two pad columns of the 192 interior rows
            g.memset(xpg[:, 0:GUARD + WP], 0.0)
            g.memset(xpg[:, GUARD + (WP - 1) * WP:2 * GUARD + NPAD], 0.0)
            xp_padcols = xpg[:, GUARD + WP:GUARD + (WP - 1) * WP].rearrange(
                "c (r w) -> c r w", w=WP)
            g.memset(xp_padcols[:, :, 0:1], 0.0)
            g.memset(xp_padcols[:, :, WP - 1:WP], 0.0)
            xp3 = xpg[:, GUARD:GUARD + NPAD].rearrange("c (r w) -> c r w", w=WP)
            # view with one extra (guard) row above and below
            xp3g = xpg[:, GUARD - WP:GUARD + NPAD + WP].rearrange(
                "c (r w) -> c r w", w=WP)

            # ---- pass A: load f32, cast to resident bf16, reduce ---------
            THA = 8
            NSA = H // THA
            maxbuf = gpool.tile([C, NSA], F32)
            sumbuf = gpool.tile([C, NSA], F32)
            for s in range(NSA):
                xa = xa_pool.tile([C, THA, W], F32)
                sy.dma_start(xa[:], x[:, s * THA:(s + 1) * THA, :])
                sc.activation(
                    xp3[:, s * THA + 1:(s + 1) * THA + 1, 1:W + 1], xa[:],
                    ACT.Copy, accum_out=sumbuf[:, s:s + 1],
                )
                v.tensor_reduce(maxbuf[:, s:s + 1], xa[:], AX.XY, ALU.max)
            maxv = gpool.tile([C, 1], F32)
            v.tensor_reduce(maxv[:], maxbuf[:], AX.X, ALU.max)
            sumv = gpool.tile([C, 1], F32)
            v.tensor_reduce(sumv[:], sumbuf[:], AX.X, ALU.add)
            pooled = gpool.tile([C, 1], F32)
            v.scalar_tensor_tensor(
                pooled[:], sumv[:], 1.0 / (H * W), maxv[:], ALU.mult, ALU.add
            )

            # ---- gate -----------------------------------------------------
            psg = ps_pool.tile([2 * E, 1], F32)
            nc.tensor.matmul(psg[:], wfc_sb[:], pooled[:], start=True, stop=True)
            g12 = gpool.tile([2 * E, 1], F32)
            v.tensor_copy(g12[:], psg[:])
            grow = gpool.tile([1, 2 * E], F32)
            sy.dma_start(grow[:], g12[:])          # partition -> free transpose
            gb = gpool.tile([1, 2 * E], F32)
            v.tensor_add(gb[:], grow[:], bfc_sb[:])
            g_pre = gb[:, 0:E]
            n_pre = gb[:, E:2 * E]

            # leaky relu(0.2)
            gl = gpool.tile([1, E], F32)
            t6 = gpool.tile([1, E], F32)
            v.tensor_scalar_mul(t6[:], g_pre, 0.2)
            v.tensor_max(gl[:], g_pre, t6[:])
            # softplus(x) = ln(1 + exp(x))
            e1 = gpool.tile([1, E], F32)
            sc.activation(e1[:], n_pre, ACT.Exp)
            noise = gpool.tile([1, E], F32)
            sc.activation(noise[:], e1[:], ACT.Ln, bias=1.0)
            # mean / unbiased std over experts
            mu = gpool.tile([1, 1], F32)
            v.tensor_reduce(mu[:], noise[:], AX.X, ALU.add)
            v.tensor_scalar_mul(mu[:], mu[:], 1.0 / E)
            d = gpool.tile([1, E], F32)
            v.tensor_scalar(d[:], noise[:], mu[:], None, ALU.subtract)
            dd = gpool.tile([1, E], F32)
            v.tensor_mul(dd[:], d[:], d[:])
            var = gpool.tile([1, 1], F32)
            v.tensor_reduce(var[:], dd[:], AX.X, ALU.add)
            v.tensor_scalar_mul(var[:], var[:], 1.0 / (E - 1))
            # 1/sqrt(var) via exp(-0.5 ln var) + one Newton step
            lnv = gpool.tile([1, 1], F32)
            sc.activation(lnv[:], var[:], ACT.Ln)
            isd0 = gpool.tile([1, 1], F32)
            sc.activation(isd0[:], lnv[:], ACT.Exp, scale=-0.5)
            ii = gpool.tile([1, 1], F32)
            v.tensor_mul(ii[:], isd0[:], isd0[:])
            v.tensor_mul(ii[:], ii[:], var[:])
            v.tensor_scalar(ii[:], ii[:], -0.5, 1.5, ALU.mult, ALU.add)
            isd = gpool.tile([1, 1], F32)
            v.tensor_mul(isd[:], isd0[:], ii[:])
            scores = gpool.tile([1, E], F32)
            v.scalar_tensor_tensor(scores[:], d[:], isd[:], gl[:], ALU.mult, ALU.add)

            # rank each expert
            ranks = gpool.tile([1, E], F32)
            cmp = gpool.tile([1, E], F32)
            for e in range(E):
                v.tensor_scalar(
                    cmp[:], scores[:], scores[0:1, e:e + 1], None, ALU.is_gt
                )
                v.tensor_reduce(ranks[:, e:e + 1], cmp[:], AX.X, ALU.add)
            mask = gpool.tile([1, E], F32)
            v.tensor_scalar(mask[:], ranks[:], float(K), None, ALU.is_lt)

            # softmax over selected: gm = (gl+30)*mask - 30
            gm = gpool.tile([1, E], F32)
            v.scalar_tensor_tensor(gm[:], gl[:], 30.0, mask[:], ALU.add, ALU.mult)
            v.tensor_scalar_sub(gm[:], gm[:], 30.0)
            gmax = gpool.tile([1, 1], F32)
            v.tensor_reduce(gmax[:], gm[:], AX.X, ALU.max)
            ngmax = gpool.tile([1, 1], F32)
            v.tensor_scalar_mul(ngmax[:], gmax[:], -1.0)
            ex = gpool.tile([1, E], F32)
            sc.activation(ex[:], gm[:], ACT.Exp, bias=ngmax[:])
            ssum = gpool.tile([1, 1], F32)
            v.tensor_reduce(ssum[:], ex[:], AX.X, ALU.add)
            rs = gpool.tile([1, 1], F32)
            v.reciprocal(rs[:], ssum[:])

            # cat = [cof(6) | slotmask(18) | cof*slotmask(18)] on partition 0
            cat = gpool.tile([1, E + 2 * K * E], F32)
            cof = cat[:, 0:E]
            v.tensor_scalar(cof, ex[:], rs[:], None, ALU.mult)
            for s in range(K):
                sm = cat[:, E + s * E:E + (s + 1) * E]
                v.tensor_scalar(sm, ranks[:], float(s), None, ALU.is_equal)
                v.tensor_mul(cat[:, E + K * E + s * E:E + K * E + (s + 1) * E], sm, cof)

            # broadcast cat to all partitions via ones[1,C].T @ cat[1,42]
            ps_bc = ps_pool.tile([C, E + 2 * K * E], F32)
            nc.tensor.matmul(ps_bc[:], ones96[:], cat[:], start=True, stop=True)
            bc = cpool.tile([C, E + 2 * K * E], F32)
            v.tensor_copy(bc[:], ps_bc[:])

            def smask_ap(s, e):      # slot-mask broadcast column
                return bc[:, E + s * E + e:E + s * E + e + 1]

            def csmask_ap(s, e):     # cof * slot-mask broadcast column
                return bc[:, E + K * E + s * E + e:E + K * E + s * E + e + 1]

            # gather slot weights: w1s (f32), w2s (bf16, cof-scaled), b1s
            w1s = cpool.tile([C, K * 9], F32)
            w2s = cpool.tile([C, K * 9], F32)
            b1s = cpool.tile([C, K], F32)
            for s in range(K):
                for e in range(E):
                    i0 = w1_sb[:, e * 9:(e + 1) * 9]
                    o0 = w1s[:, s * 9:(s + 1) * 9]
                    if e == 0:
                        v.tensor_scalar(o0, i0, smask_ap(s, e), None, ALU.mult)
                    else:
                        v.scalar_tensor_tensor(o0, i0, smask_ap(s, e), o0,
                                               ALU.mult, ALU.add)
                    i2 = w2_sb[:, e * 9:(e + 1) * 9]
                    o2 = w2s[:, s * 9:(s + 1) * 9]
                    if e == 0:
                        v.tensor_scalar(o2, i2, csmask_ap(s, e), None, ALU.mult)
                    else:
                        v.scalar_tensor_tensor(o2, i2, csmask_ap(s, e), o2,
                                               ALU.mult, ALU.add)
                    ib = b1_sb[:, e:e + 1]
                    ob = b1s[:, s:s + 1]
                    if e == 0:
                        v.tensor_scalar(ob, ib, smask_ap(s, e), None, ALU.mult)
                    else:
                        v.scalar_tensor_tensor(ob, ib, smask_ap(s, e), ob,
                                               ALU.mult, ALU.add)
            # b2tot = sum_e cof_e * b2_e
            tb = gpool.tile([C, E], F32)
            v.tensor_mul(tb[:], b2_sb[:], bc[:, 0:E])
            b2tot = cpool.tile([C, 1], F32)
            v.tensor_reduce(b2tot[:], tb[:], AX.X, ALU.add)

            # diagonal weight matrices for conv1-on-PE: diag1[c, k, :] = w1s[c,k] * eye[c,:]
            diag1 = cpool.tile([C, K * 9, C], BF)
            for k in range(K * 9):
                v.tensor_scalar(diag1[:, k, :], eye_sb[:], w1s[:, k:k + 1],
                                None, ALU.mult)
            eye_bf = cpool.tile([C, C], BF)
            v.tensor_copy(eye_bf[:], eye_sb[:])

            # ---- pass B ---------------------------------------------------
            HFLAT = (TH + 2) * WP
            n_chunks = (HFLAT + CHUNK - 1) // CHUNK
            DB = GUARD  # flat base of padded image
            # engine split knobs (module-level for tuning)
            C1_GP = dict(CONF_C1_GP)     # conv1 trailing taps on GpSimd, per slot
            C2_GP = dict(CONF_C2_GP)     # conv2 trailing taps on GpSimd, per slot
            C2_ACT = dict(CONF_C2_ACT)   # conv2 taps whose multiply runs on ACT
            for s in range(NS):
                h0 = s * TH
                oacc = oacc_pool.tile([C, TH, W], BF)
                oaccg = og_pool.tile([C, TH, W], BF, tag="oaccg")
                first_dve = True
                first_gp = True
                for slot in range(K):
                    n_gp1 = C1_GP[slot]
                    ht = h_pool.tile([C, TH + 2, WP], BF)
                    hf = ht[:].rearrange("c r w -> c (r w)")
                    # gpsimd partial of conv1 (accumulated in sbuf, merged
                    # into psum via an eye-matmul)
                    h2f = None
                    if n_gp1:
                        h2 = h2_pool.tile([C, TH + 2, WP], BF, tag="h2")
                        h2f = h2[:].rearrange("c r w -> c (r w)")
                        g.memset(h2[:, :, 0:1], 0.0)
                        g.memset(h2[:, :, WP - 1:WP], 0.0)
                        for j, (dy, dx) in enumerate(TAPS[9 - n_gp1:]):
                            it = 9 - n_gp1 + j
                            # guarded 3D view: rows shifted by +1 (covers row -1)
                            in0 = xp3g[:, h0 + dy + 1:h0 + dy + 1 + TH + 2,
                                       1 + dx:1 + dx + W]
                            wap = w1s[:, slot * 9 + it:slot * 9 + it + 1]
                            out2 = h2[:, :, 1:W + 1]
                            if j == 0:
                                g.tensor_scalar(out2, in0, wap, None, ALU.mult)
                            else:
                                pg1 = h_pool.tile([C, TH + 2, W], BF, tag="ptmpg1")
                                g.tensor_scalar(pg1[:], in0, wap, None, ALU.mult)
                                g.tensor_add(out2, out2, pg1[:])
                    for ci in range(n_chunks):
                        a0 = ci * CHUNK
                        csz = min(CHUNK, HFLAT - a0)
                        ps = psc_pool.tile([C, CHUNK], F32, tag="convps")
                        n_pe = 9 - n_gp1
                        for it in range(n_pe):
                            dy, dx = TAPS[it]
                            delta = dy * WP + dx
                            rhs = xpg[:, DB + h0 * WP + a0 + delta:
                                      DB + h0 * WP + a0 + delta + csz]
                            nc.tensor.matmul(
                                ps[:, 0:csz], diag1[:, 9 * slot + it, :], rhs,
                                start=(it == 0), stop=(it == n_pe - 1 and not n_gp1),
                            )
                        if n_gp1:
                            nc.tensor.matmul(
                                ps[:, 0:csz], eye_bf[:], h2f[:, a0:a0 + csz],
                                start=False, stop=True,
                            )
                        sc.activation(hf[:, a0:a0 + csz], ps[:, 0:csz],
                                      ACT.Relu, bias=b1s[:, slot:slot + 1])
                    # zero h padding (cols, and top/bottom edge rows)
                    g.memset(ht[:, :, 0:1], 0.0)
                    g.memset(ht[:, :, WP - 1:WP], 0.0)
                    if s == 0:
                        g.memset(ht[:, 0:1, :], 0.0)
                    if s == NS - 1:
                        g.memset(ht[:, TH + 1:TH + 2, :], 0.0)

                    # conv2: DVE (ts-mul + tt-add pairs) / GpSimd (fused STT)
                    n_gp2 = C2_GP[slot]
                    n_act2 = C2_ACT[slot]
                    for it, (dy, dx) in enumerate(TAPS):
                        in0 = ht[:, 1 + dy:1 + dy + TH, 1 + dx:1 + dx + W]
                        wap = w2s[:, slot * 9 + it:slot * 9 + it + 1]
                        if 9 - n_gp2 - n_act2 <= it < 9 - n_gp2 and not first_dve:
                            # multiply on ACT (idle capacity), add on DVE
                            p = h_pool.tile([C, TH, W], BF, tag="ptmp")
                            sc.activation(p[:], in0, ACT.Copy, scale=wap)
                            v.tensor_add(oacc[:], oacc[:], p[:])
                        elif it >= 9 - n_gp2:
                            # walrus rejects STT on Pool; use ts-mul + tt-add
                            if first_gp:
                                g.tensor_scalar(oaccg[:], in0, wap, None, ALU.mult)
                                first_gp = False
                            else:
                                pg = h_pool.tile([C, TH, W], BF, tag="ptmpg")
                                g.tensor_scalar(pg[:], in0, wap, None, ALU.mult)
                                g.tensor_add(oaccg[:], oaccg[:], pg[:])
                        else:
                            if first_dve:
                                v.tensor_scalar(oacc[:], in0, wap, None, ALU.mult)
                                first_dve = False
                            else:
                                p = h_pool.tile([C, TH, W], BF, tag="ptmp")
                                v.tensor_scalar(p[:], in0, wap, None, ALU.mult)
                                v.tensor_add(oacc[:], oacc[:], p[:])
                # merge accumulators (DVE), bias + f32 cast (ACT), store (HWDGE)
                v.tensor_add(oacc[:], oacc[:], oaccg[:])
                of32 = of32_pool.tile([C, TH, W], F32)
                sc.activation(of32[:], oacc[:], ACT.Identity, bias=b2tot[:])
                sy.dma_start(y[:, h0:h0 + TH, :], of32[:])

    if split:
        _split_multiwait(nc, maxw=1)
    return nc


_NC_CACHE = {}


def _get_nc():
    if "nc" not in _NC_CACHE:
        _NC_CACHE["nc"] = _build()
    return _NC_CACHE["nc"]


class _Runner:
    """Compile-once SPMD runner (mirrors bass2jax.run_bass_via_pjrt's
    multi-core path, but keeps the jitted executable for reuse/benching)."""

    def __init__(self, nc, n_cores):
        import jax
        from jax.experimental.shard_map import shard_map
        from jax.sharding import Mesh, PartitionSpec
        from concourse import bass2jax, mybir as _mybir

        bass2jax.install_neuronx_cc_hook()
        self.jax = jax
        partition_name = (
            nc.partition_id_tensor.name if nc.partition_id_tensor else None
        )
        in_names, out_names, out_avals, zero_outs = [], [], [], []
        for alloc in nc.m.functions[0].allocations:
            if not isinstance(alloc, _mybir.MemoryLocationSet):
                continue
            name = alloc.memorylocations[0].name
            if alloc.kind == "ExternalInput":
                if name == partition_name:
                    continue
                in_names.append(name)
            elif alloc.kind == "ExternalOutput":
                shape = tuple(alloc.tensor_shape)
                dtype = _mybir.dt.np(alloc.dtype)
                out_names.append(name)
                out_avals.append(jax.core.ShapedArray(shape, dtype))
                zero_outs.append(np.zeros(shape, dtype))
        self.in_names, self.out_names = in_names, out_names
        self.out_avals, self.zero_outs = out_avals, zero_outs
        n_params, n_outs = len(in_names), len(out_names)
        self.n_cores = n_cores
        donate = tuple(range(n_params, n_params + n_outs))

        all_in_names = in_names + out_names
        if partition_name is not None:
            all_in_names = all_in_names + [partition_name]

        def _body(*args):
            operands = list(args)
            if partition_name is not None:
                operands.append(bass2jax.partition_id_tensor())
            outs = bass2jax._bass_exec_p.bind(
                *operands,
                out_avals=tuple(out_avals),
                in_names=tuple(all_in_names),
                out_names=tuple(out_names),
                lowering_input_output_aliases=(),
                sim_require_finite=True,
                sim_require_nnan=True,
                nc=nc,
            )
            return tuple(outs)

        devices = jax.devices()[:n_cores]
        mesh = Mesh(np.asarray(devices), ("core",))
        self.sharded = jax.jit(
            shard_map(
                _body,
                mesh=mesh,
                in_specs=(PartitionSpec("core"),) * (n_params + n_outs),
                out_specs=(PartitionSpec("core"),) * n_outs,
                check_rep=False,
            ),
            donate_argnums=donate,
            keep_unused=True,
        )

    def concat_inputs(self, in_maps):
        return [
            np.concatenate([np.asarray(m[name]) for m in in_maps], axis=0)
            for name in self.in_names
        ]

    def concat_zeros(self):
        return [
            np.zeros((self.n_cores * z.shape[0], *z.shape[1:]), z.dtype)
            for z in self.zero_outs
        ]

    def run(self, in_maps):
        out_arrs = self.sharded(*self.concat_inputs(in_maps), *self.concat_zeros())
        return [
            {
                name: np.asarray(out_arrs[i]).reshape(
                    self.n_cores, *self.out_avals[i].shape
                )[c]
                for i, name in enumerate(self.out_names)
            }
            for c in range(self.n_cores)
        ]


def _get_runner():
    if "runner" not in _NC_CACHE:
        _NC_CACHE["runner"] = _Runner(_get_nc(), N_CORES)
    return _NC_CACHE["runner"]


_EYE = np.ascontiguousarray(np.eye(C, dtype=np.float32))


def kernel(x, w_fc0, b_fc0, w_fc1, b_fc1, ew1, eb1, ew2, eb2):
    x = np.asarray(x, dtype=np.float32)
    f32 = lambda a: np.ascontiguousarray(np.asarray(a, dtype=np.float32))
    wfc = f32(np.concatenate([np.asarray(w_fc1).T, np.asarray(w_fc0).T], axis=1))
    bfc = f32(np.concatenate([np.asarray(b_fc1), np.asarray(b_fc0)])[None, :])
    w1p = f32(np.asarray(ew1).reshape(E, C, 9).transpose(1, 0, 2).reshape(C, E * 9))
    b1p = f32(np.asarray(eb1).T)
    w2p = f32(np.asarray(ew2).reshape(E, C, 9).transpose(1, 0, 2).reshape(C, E * 9))
    b2p = f32(np.asarray(eb2).T)

    in_maps = []
    for b in range(B):
        in_maps.append({
            "x": np.ascontiguousarray(x[b]),
            "wfc": wfc, "bfc": bfc,
            "w1": w1p, "b1": b1p, "w2": w2p, "b2": b2p,
            "eye": _EYE,
        })
    res = _get_runner().run(in_maps)
    out = np.stack([res[b]["y"] for b in range(B)], axis=0)
    return out.astype(np.float32)


if __name__ == "__main__":
    data = np.load("/tmp/ref_data.npz")
    inputs = {k: data[k] for k in
              ["x", "w_fc0", "b_fc0", "w_fc1", "b_fc1", "ew1", "eb1", "ew2", "eb2"]}
    out = kernel(**inputs)
    exp = data["out"]
    err = np.linalg.norm(out - exp) / np.linalg.norm(exp)
    print("Relative error:", err)
    print("max abs diff:", np.abs(out - exp).max())

